# revision 21
# baseline (speedup 1.0000x reference)
"""Affine-transform + bidirectional Chamfer loss on 8 Trainium2 NeuronCores.

reference math:
    x = pts_mov @ mat[0] + trans[0].T          # [N,3]
    d2[i,j] = ||x_i - y_j||^2 = x2_i + y2_j - 2 x_i.y_j   (clamped at 0)
    out = mean_i min_j d2 + mean_j min_i d2

Strategy:
  - Shard the 16384 moving points across the 8 cores (2048 rows each).
  - The whole distance computation is folded into bf16 matmuls:
    stationary lhsT [K=24, 128 x-points], moving rhs [K=24, 512 y-points],
    out[i,j] = sum_k lhsT[k,i]*rhs[k,j] = x2_i + y2_j - 2 x_i.y_j.
    fp32 accuracy is recovered from bf16 operands by 3-way hi/mid/lo
    splitting of each coordinate (products accumulate exactly in fp32 PSUM).
  - K=24 <= 32, so operands are replicated into four 32-row partition groups
    and each [128, 2048] PSUM group-tile is produced by 4 row-group-packed
    matmuls (tile_position=(32r,0)) that run concurrently on the PE array --
    ~4x the column rate of a single matmul and insensitive to HAM throttle.
  - Per group-tile, each engine makes exactly one 1-elem/lane/cycle pass:
      DVE  tensor_reduce(min, free axis) -> row-min partials (moving->fixed)
      ACT  tensor_tensor(min) from PSUM  -> column-min candidates
    which is the PSUM-port-bandwidth floor for touching every distance twice.
  - Column mins leave the device as a [128, 16384] bf16 candidate matrix per
    core (min over x-tiles only); the final min over (partition, core), the
    clamps, and the two means run on host - O(N) work.
"""

import os
import sys

import numpy as np

for _p in ("/opt/trn_rl_repo",):
    if _p not in sys.path and os.path.isdir(_p):
        sys.path.append(_p)

import ml_dtypes

BF16 = ml_dtypes.bfloat16

N_MOV = 16384
N_FIX = 16384
N_CORES = 8
NSH = N_MOV // N_CORES    # 2048 moving points per core
P = 128                   # partitions
XT = NSH // P             # 16 x-tiles per core
GW = 2048                 # j-group width (4 PSUM banks)
NG = N_FIX // GW          # 8 j-groups
BANK = 512                # fp32 elems per PSUM bank
K = 24                    # contraction rows (see _build_operands)
KP = 120                  # 4 replicas at partition offsets 0/32/64/96

_CACHE = {}


def _split3(v):
    """3-term bf16 decomposition of fp32 array: v ~= h + m + l (residual ~2^-26|v|)."""
    h = v.astype(BF16).astype(np.float32)
    r = (v - h).astype(np.float32)
    m = r.astype(BF16).astype(np.float32)
    l = (r - m).astype(np.float32).astype(BF16)
    return h.astype(BF16), m.astype(BF16), l


def _build_operands(x, y):
    """lhsT [K, N] / rhs [K, M] bf16 such that lhsT.T @ rhs = x2 + y2 - 2 x.y.

    Kept product pairs per coordinate d: (h,h),(h,m),(m,h),(h,l),(l,h),(m,m)
    -> 18 rows; plus x2 (3-split) against ones and ones against y2 (3-split)
    -> 6 rows. Dropped terms are O(2^-25 |x||y|).
    """
    n, m = x.shape[0], y.shape[0]
    x2 = np.sum(x * x, axis=1, dtype=np.float32)
    y2 = np.sum(y * y, axis=1, dtype=np.float32)

    lhs = np.zeros((K, n), dtype=BF16)
    rhs = np.zeros((K, m), dtype=BF16)
    row = 0
    for d in range(3):
        xh, xm, xl = _split3(x[:, d])
        yh, ym, yl = _split3(y[:, d])
        n2yh = (-2.0 * yh.astype(np.float32)).astype(BF16)  # exact: power-of-2 scale
        n2ym = (-2.0 * ym.astype(np.float32)).astype(BF16)
        n2yl = (-2.0 * yl.astype(np.float32)).astype(BF16)
        for xa, yb in ((xh, n2yh), (xh, n2ym), (xm, n2yh),
                       (xh, n2yl), (xl, n2yh), (xm, n2ym)):
            lhs[row] = xa
            rhs[row] = yb
            row += 1
    x2h, x2m, x2l = _split3(x2)
    y2h, y2m, y2l = _split3(y2)
    ones_n = np.ones(n, dtype=BF16)
    ones_m = np.ones(m, dtype=BF16)
    for xa, yb in ((x2h, ones_m), (x2m, ones_m), (x2l, ones_m),
                   (ones_n, y2h), (ones_n, y2m), (ones_n, y2l)):
        lhs[row] = xa
        rhs[row] = yb
        row += 1
    assert row == K
    return lhs, rhs


def _replicate4(a):
    """[K, n] -> [KP, n] with copies at partition offsets 0/32/64/96."""
    out = np.zeros((KP, a.shape[1]), dtype=a.dtype)
    for r in range(4):
        out[32 * r:32 * r + K] = a
    return out


def build_nc(nsh=NSH, n_fix=N_FIX, n_cores=N_CORES, gps_share=5):
    """Build + compile the SPMD Bass program (same program on every core)."""
    from contextlib import ExitStack

    import concourse.bass as bass  # noqa: F401
    import concourse.tile as tile
    from concourse import bacc, mybir

    xt_n = nsh // P
    ng = n_fix // GW
    f32 = mybir.dt.float32
    bf16 = mybir.dt.bfloat16
    AOP = mybir.AluOpType
    RELU = mybir.ActivationFunctionType.Relu

    nc = bacc.Bacc("TRN2", target_bir_lowering=False, debug=False,
                   num_devices=n_cores)
    lhs_d = nc.dram_tensor("lhs", [KP, nsh], bf16, kind="ExternalInput").ap()
    rhs_d = nc.dram_tensor("rhs", [KP, n_fix], bf16, kind="ExternalInput").ap()
    rowmin_d = nc.dram_tensor("rowmin", [P, xt_n], f32, kind="ExternalOutput").ap()
    colmin_d = nc.dram_tensor("colmin", [P, n_fix], bf16, kind="ExternalOutput").ap()

    with tile.TileContext(nc) as tc, ExitStack() as ctx:
        const = ctx.enter_context(tc.tile_pool(name="const", bufs=1))
        rhs_pool = ctx.enter_context(tc.tile_pool(name="rhsp", bufs=1))
        rowp = ctx.enter_context(tc.tile_pool(name="rowp", bufs=1))
        psum = ctx.enter_context(
            tc.tile_pool(name="psum", bufs=2, space=bass.MemorySpace.PSUM))
        tpool = ctx.enter_context(tc.tile_pool(name="tp", bufs=3))
        colp = ctx.enter_context(tc.tile_pool(name="colacc", bufs=2))

        # The opening loads (lhs + first rhs group) are split into column
        # chunks so they spread over many DMA engines; later rhs groups are
        # prefetched two groups ahead (dependency-staged) so the opening
        # matmuls aren't starved by DMA packet round-robin over all inputs.
        lhs_sb = const.tile([KP, nsh], bf16)
        nq = max(1, nsh // 512)
        for q in range(nq):
            w = nsh // nq
            nc.sync.dma_start(lhs_sb[:, q * w:(q + 1) * w],
                              lhs_d[:, q * w:(q + 1) * w])
        rowmin_sb = const.tile([P, xt_n], f32)

        rhs_sb = []
        for g in range(ng):
            r = rhs_pool.tile([KP, GW], bf16, tag=f"rhs{g}")
            if g == 0:
                for q in range(4):
                    nc.sync.dma_start(r[:, q * BANK:(q + 1) * BANK],
                                      rhs_d[:, q * BANK:(q + 1) * BANK])
            rhs_sb.append(r)
        rowaccs = []
        for xt in range(xt_n):
            rowacc = rowp.tile([P, GW], bf16, tag=f"rowacc{xt}")
            rowaccs.append(rowacc)

        for g in range(ng):
            colacc = colp.tile([P, GW], bf16)
            for xt in range(xt_n):
                ps = psum.tile([P, GW], f32)
                mm0 = None
                for r in range(4):
                    mm = nc.tensor.matmul(
                        ps[:, r * BANK:(r + 1) * BANK],
                        lhs_sb[32 * r:32 * r + K, xt * P:(xt + 1) * P],
                        rhs_sb[g][32 * r:32 * r + K, r * BANK:(r + 1) * BANK],
                        start=True, stop=True, tile_position=(32 * r, 0))
                    mm0 = mm0 or mm
                prefetch = []
                if xt == 0:
                    if g == 0:
                        prefetch = [1, 2]
                    elif g + 2 < ng:
                        prefetch = [g + 2]
                for gp in prefetch:
                    dma = nc.sync.dma_start(
                        rhs_sb[gp][:], rhs_d[:, gp * GW:(gp + 1) * GW])
                    try:
                        bass._add_dep_helper(
                            dma.ins, mm0.ins, sync=True,
                            reason="stage rhs prefetch behind current group")
                    except Exception:
                        pass
                # ACT: the single PSUM drain (clamp + fp32->bf16). The first
                # drain of a row/column chain lands directly in the
                # accumulator tile (no init copy); the other direction's
                # min-update then reads the accumulator as its source.
                if g == 0:
                    dst = rowaccs[xt]
                elif xt == 0:
                    dst = colacc
                else:
                    dst = tpool.tile([P, GW], bf16, tag="t")
                nc.scalar.activation(dst[:], ps[:], RELU)
                # DVE row-min accumulation across groups (bf16, 2x mode).
                if g > 0:
                    nc.vector.tensor_tensor(
                        rowaccs[xt][:], rowaccs[xt][:], dst[:], AOP.min)
                # DVE column-min accumulation across x-tiles (bf16, 2x mode).
                if xt > 0:
                    nc.vector.tensor_tensor(
                        colacc[:], colacc[:], dst[:], AOP.min)
                elif g == 0:
                    nc.vector.tensor_copy(colacc[:], dst[:])
            nc.sync.dma_start(colmin_d[:, g * GW:(g + 1) * GW], colacc[:])

        # Row-min tail: 2x tt-tree then a short 1x reduce per x-tile.
        for xt in range(xt_n):
            ra = rowaccs[xt]
            half = GW // 2
            while half >= 256:
                nc.vector.tensor_tensor(
                    ra[:, 0:half], ra[:, 0:half], ra[:, half:2 * half], AOP.min)
                half //= 2
            nc.vector.tensor_reduce(
                rowmin_sb[:, xt:xt + 1], ra[:, 0:2 * half],
                axis=mybir.AxisListType.X, op=AOP.min)
        nc.sync.dma_start(rowmin_d[:], rowmin_sb[:])

    nc.compile()
    return nc


def _get_nc():
    if "nc" not in _CACHE:
        _CACHE["nc"] = build_nc()
    return _CACHE["nc"]


def make_in_maps(pts_fixed, pts_mov, mat, trans):
    """Host prep: affine transform (fp32, mirrors reference) + operand build."""
    x = (pts_mov.astype(np.float32) @ mat[0].astype(np.float32)
         + trans[0].astype(np.float32).T).astype(np.float32)
    y = pts_fixed.astype(np.float32)
    lhs, rhs = _build_operands(x, y)          # lhs [K, N_MOV], rhs [K, N_FIX]
    lhs4, rhs4 = _replicate4(lhs), _replicate4(rhs)
    return [{"lhs": np.ascontiguousarray(lhs4[:, c * NSH:(c + 1) * NSH]),
             "rhs": rhs4} for c in range(N_CORES)]


def combine_outputs(results):
    rowmins = np.concatenate(
        [np.asarray(r["rowmin"], dtype=np.float32).ravel() for r in results])
    colmins = np.stack(
        [np.asarray(r["colmin"]).astype(np.float32) for r in results])  # [8,128,M]
    rowmins = np.maximum(rowmins, 0.0)
    colmin = np.maximum(colmins.min(axis=(0, 1)), 0.0)                  # [M]
    cham_mov = np.float32(rowmins.mean(dtype=np.float64))
    cham_fix = np.float32(colmin.mean(dtype=np.float64))
    return np.array(np.float32(cham_mov + cham_fix))


def run_on_device(in_maps, trace=False, **kw):
    from concourse.bass_utils import run_bass_kernel_spmd
    nc = _get_nc()
    return run_bass_kernel_spmd(nc, in_maps, list(range(N_CORES)),
                                trace=trace, **kw)


def kernel(pts_fixed, pts_mov, mat, trans):
    in_maps = make_in_maps(np.asarray(pts_fixed), np.asarray(pts_mov),
                           np.asarray(mat), np.asarray(trans))
    res = run_on_device(in_maps)
    return combine_outputs(res.results)
